# revision 2
# baseline (speedup 1.0000x reference)
"""Multi-head attention (16 heads, d_model=1024, head_dim=64) on 8 trn2 cores.

Sharding: core c handles batch b = c//2 and heads [8*(c%2), 8*(c%2)+8)
(data parallel over batch x tensor parallel over heads). Each core
computes its 8 heads' Q/K/V projections, attention, and a partial output
projection; the host sums the two partial projections per batch element
(the "all-reduce") and adds the output bias bp.

Key layout choices (all driven by PE cost ~ moving-operand columns):
- Q^T/K^T are produced feature-major [d, t] so energies contract along
  partitions (stationary K^T chunk [64, 128], moving Q^T [64, 512]).
- attn@V is computed with the EXP WEIGHTS AS STATIONARY ([128 keys, 128
  queries]) and V as the moving operand ([128 keys, 65]): each matmul
  streams only 65 columns (64 head dims + a ones-column that yields the
  softmax row-sums), 8x fewer moving columns than the [d, t]-stationary
  orientation. The attention output lands QUERY-major [q, d], where the
  softmax normalization is a cheap per-partition scalar multiply.
- Normalized [q, f] tiles are transposed back to feature-major via PE
  transpose (128 cycles per 128x128 tile) to feed the output projection;
  the V bias is folded into the transpose-evacuation copy (rows of the
  normalized attention matrix sum to 1, so A@(V + bv) = A@V + bv).

Softmax is unnormalized exp (no max subtraction; energies bounded ~|15|)
with row sums taken by the ones-column of V. exp runs on the Activation
engine in (3,3,3,3,3,1)-key-chunk calls - the largest granularity that
fits PSUM: 2x3 banks of energies ping-pong + 1 bank attn@V accumulator +
1 bank for projection/transpose scratch.
"""

import numpy as np
import ml_dtypes

from concourse import bass, bacc, tile, mybir
from concourse.bass_utils import run_bass_kernel_spmd

BF16 = ml_dtypes.bfloat16
dt = mybir.dt
AF = mybir.ActivationFunctionType

N_CORES = 8
T = 2048          # tokens per batch element
D = 1024          # model dim
FH = 512          # features (head dims) per core: 8 heads x 64
NH_LOC = 8        # heads per core
HD = 64           # head dim

_prog_cache = {}


def _build_program():
    nc = bacc.Bacc("TRN2", target_bir_lowering=False, debug=False,
                   num_devices=N_CORES)

    xT = nc.dram_tensor("xT", [D, T], dt.bfloat16, kind="ExternalInput").ap()
    wqT = nc.dram_tensor("wqT", [D, FH], dt.bfloat16, kind="ExternalInput").ap()
    wkT = nc.dram_tensor("wkT", [D, FH], dt.bfloat16, kind="ExternalInput").ap()
    wvT = nc.dram_tensor("wvT", [D, FH], dt.bfloat16, kind="ExternalInput").ap()
    bqkv = nc.dram_tensor("bqkv", [128, 12], dt.float32, kind="ExternalInput").ap()
    wpT = nc.dram_tensor("wpT", [FH, D], dt.bfloat16, kind="ExternalInput").ap()
    iden = nc.dram_tensor("iden", [128, 128], dt.bfloat16, kind="ExternalInput").ap()
    out = nc.dram_tensor("out", [T, D], dt.bfloat16, kind="ExternalOutput").ap()

    with tile.TileContext(nc) as tc:
        _emit(tc, out, xT, wqT, wkT, wvT, bqkv, wpT, iden)
    nc.compile()
    return nc


def _emit(tc, out, xT, wqT, wkT, wvT, bqkv, wpT, iden):
    nc = tc.nc
    f32 = dt.float32
    bf16 = dt.bfloat16

    with (
        tc.tile_pool(name="sbp", bufs=1) as sbp,
        tc.tile_pool(name="qkv_sb", bufs=1) as qkv_sb,
        tc.tile_pool(name="pb_pool", bufs=4) as pb_pool,
        tc.tile_pool(name="ao_pool", bufs=1) as ao_pool,
        tc.tile_pool(name="rr_pool", bufs=2) as rr_pool,
        tc.tile_pool(name="ostage", bufs=2) as ostage,
        # PSUM budget (8 banks): energies 3x[128,2,512] (6 banks) rotating
        # under the exp stream; attn@V accumulator bank (also used by half
        # the V-projection tiles at the head, before attn@V starts); one
        # shared bank for QK/V/proj chains and transpose evacuation.
        tc.tile_pool(name="ps_e", bufs=3, space="PSUM") as ps_e,
        tc.tile_pool(name="ps_av", bufs=1, space="PSUM") as ps_av,
        tc.tile_pool(name="ps_m", bufs=1, space="PSUM") as ps_m,
    ):
        # ---- input DMAs. The DMA engines share one per-core bandwidth
        # pool (~350 GB/s), so what matters is BYTE order, not queue count:
        # the K^T(0)/Q^T(0) n=0 tiles only touch tokens 0..511, so x is
        # loaded in token-quarters and the first exp can start after just
        # wk + x-quarter0 + wq (~3 MB) instead of all 8 MB of inputs.
        x_s = sbp.tile([128, 8, T], bf16)
        xr = xT.rearrange("(m p) t -> p m t", p=128)
        wk_s = sbp.tile([128, 8, FH], bf16, tag="wk")
        nc.sync.dma_start(out=wk_s[:], in_=wkT.rearrange("(m p) d -> p m d", p=128))
        nc.scalar.dma_start(out=x_s[:, :, 0:512], in_=xr[:, :, 0:512])
        wq_s = sbp.tile([128, 8, FH], bf16, tag="wq")
        nc.sync.dma_start(out=wq_s[:], in_=wqT.rearrange("(m p) d -> p m d", p=128))
        bqkv_s = sbp.tile([128, 12], f32)
        nc.scalar.dma_start(out=bqkv_s[:], in_=bqkv)
        bqT_s = bqkv_s[:, 0:4]
        bkT_s = bqkv_s[:, 4:8]
        bvT_s = bqkv_s[:, 8:12]
        nc.scalar.dma_start(out=x_s[:, :, 512:1024], in_=xr[:, :, 512:1024])
        nc.sync.dma_start(out=x_s[:, :, 1024:1536], in_=xr[:, :, 1024:1536])
        nc.scalar.dma_start(out=x_s[:, :, 1536:2048], in_=xr[:, :, 1536:2048])
        wv_s = sbp.tile([128, 8, FH], bf16, tag="wv")
        nc.sync.dma_start(out=wv_s[:], in_=wvT.rearrange("(m p) d -> p m d", p=128))
        id_s = sbp.tile([128, 128], bf16)
        nc.scalar.dma_start(out=id_s[:], in_=iden)
        wp_s = sbp.tile([128, 4, D], bf16)
        nc.scalar.dma_start(out=wp_s[:], in_=wpT.rearrange("(c p) o -> p c o", p=128))

        # QT/KT: [d-in-pair(128), head-pair(4), t]; V: [t-in-chunk(128),
        # t-chunk(16), head(8), 66] with col 64 = 1.0 (row-sum trick).
        QT_sb = qkv_sb.tile([128, 4, T], bf16)
        KT_sb = qkv_sb.tile([128, 4, T], bf16)
        V_sb = qkv_sb.tile([128, 16, NH_LOC, 65], bf16)
        nc.vector.memset(V_sb[:, :, :, 64:65], 1.0)
        # AttnOut^T (post-transpose): [f-in-chunk(128), f-chunk(4)=hp, t]
        AOT_sb = qkv_sb.tile([128, 4, T], bf16)

        def emit_qk_ntile(w_s, b_s, dst, hp, n, pool):
            # one n-tile of a Q^T/K^T projection: an 8-matmul chain + bias
            dsl = slice(hp * 128, (hp + 1) * 128)
            ps = pool.tile([128, 512], f32, tag="e" if pool is ps_e else "m",
                           name="qk_ps")
            for m in range(8):
                nc.tensor.matmul(ps[:], w_s[:, m, dsl],
                                 x_s[:, m, n * 512:(n + 1) * 512],
                                 start=(m == 0), stop=(m == 7))
            nc.vector.tensor_scalar_add(
                dst[:, hp, n * 512:(n + 1) * 512], ps[:], b_s[:, hp:hp + 1])

        def emit_v_tile(t, pool):
            # V (natural): out[t, f] = x[t, :].wvT[:, f]  (bias folded later)
            tag = {id(ps_e): "e", id(ps_av): "av", id(ps_m): "m"}[id(pool)]
            ps = pool.tile([128, 512], f32, tag=tag, name="v_ps")
            for m in range(8):
                nc.tensor.matmul(ps[:], x_s[:, m, t * 128:(t + 1) * 128],
                                 wv_s[:, m, :], start=(m == 0), stop=(m == 7))
            nc.vector.tensor_copy(
                V_sb[:, t, :, 0:64],
                ps[:].rearrange("p (h d) -> p h d", h=NH_LOC))

        def emit_proj(t, pool=None):
            # partial output projection (pre-bias) for token tile t
            pool = pool or ps_m
            tag = {id(ps_e): "e", id(ps_av): "av", id(ps_m): "m"}[id(pool)]
            tsl = slice(t * 128, (t + 1) * 128)
            st = ostage.tile([128, D], bf16, tag="st")
            for half in range(2):
                ps = pool.tile([128, 512], f32, tag=tag, name="pj")
                for fc in range(4):
                    nc.tensor.matmul(ps[:], AOT_sb[:, fc, tsl],
                                     wp_s[:, fc, half * 512:(half + 1) * 512],
                                     start=(fc == 0), stop=(fc == 3))
                nc.vector.tensor_copy(st[:, half * 512:(half + 1) * 512], ps[:])
            nc.sync.dma_start(out=out[tsl, :], in_=st[:])

        # ---- software-pipelined attention ----
        units = [(hp, j, s) for hp in range(4) for j in range(4)
                 for s in range(2)]
        state = {}      # u -> (pb, av) live tiles
        fillers = []    # queue of zero-arg emitters

        GROUPS = [(2 * g, 2 * g + 2) for g in range(8)]

        def emit_e_group(u, g):
            # fill one energies PSUM tile (3 key-chunks, 1 for the last
            # group) and exp it into pb
            hp, j, s = u
            psl = slice(64 * s, 64 * s + 64)
            qsl = slice(j * 512, (j + 1) * 512)
            pb = state[u][0][0] if g < 4 else state[u][0][1]
            lo, hi = GROUPS[g]
            off = 0 if g < 4 else 8
            w = hi - lo
            et = ps_e.tile([128, 2, 512], f32, tag="e", name="et")
            for i in range(w):
                kc = lo + i
                nc.tensor.matmul(et[:, i, :], KT_sb[psl, hp, kc * 128:(kc + 1) * 128],
                                 QT_sb[psl, hp, qsl], start=True, stop=True)
            nc.scalar.activation(pb[:, lo - off:hi - off, :], et[:, 0:w, :], AF.Exp)

        def emit_av_block(u, kcs):
            # attn@V with pb as stationary: out[q, d] accumulates per
            # q-chunk; V col 64 (ones) accumulates the softmax row sums.
            hp, j, s = u
            h = 2 * hp + s
            (pba, pbb), av = state[u][0], state[u][1]
            if av is None:
                avt = ps_av.tile([128, 512], f32, tag="av", name="avt")
                av = avt[:, 0:260].rearrange("p (q c) -> p q c", c=65)
                state[u] = ((pba, pbb), av, state[u][2])
            # One PSUM accumulation group for the WHOLE tile: start=True
            # zeroes the entire 2KB zero-region, so only the first matmul
            # emitted may carry it, and only the last carries stop (the
            # framework serializes accumulates into the same tile, so
            # emission order is execution order here).
            for qc in range(4):
                for kc in kcs:
                    pb = pba if kc < 8 else pbb
                    nc.tensor.matmul(
                        av[:, qc, :],
                        pb[:, kc % 8, qc * 128:(qc + 1) * 128],
                        V_sb[:, kc, h, 0:65],
                        start=(qc == 0 and kc == 0),
                        stop=(qc == 3 and kc == 15))

        def emit_norm(u):
            # softmax normalization: per-partition (per-query) reciprocal
            # of the row sums, then scale the 64 head dims into AO staging
            # [q, qsub, pair-features].
            hp, j, s = u
            av = state[u][1]
            ao = state[u][2]
            rr = rr_pool.tile([128, 4], f32, tag="rr", bufs=4)
            nc.vector.reciprocal(rr[:], av[:, :, 64])
            for qc in range(4):
                nc.vector.tensor_scalar_mul(
                    ao[:, qc, s * 64:(s + 1) * 64], av[:, qc, 0:64],
                    rr[:, qc:qc + 1])
            del state[u]

        def emit_transposes(hp, j, ao):
            # [q, f]-major AO pair-block back to feature-major AOT, adding
            # the V bias during the PSUM evacuation.
            for qc in range(4):
                tp = ps_m.tile([128, 128], bf16, tag="m", name="tp")
                nc.tensor.transpose(tp[:], ao[:, qc, :], id_s[:])
                nc.vector.tensor_scalar_add(
                    AOT_sb[:, hp, j * 512 + qc * 128:j * 512 + (qc + 1) * 128],
                    tp[:], bvT_s[:, hp:hp + 1])

        def pop_filler():
            if fillers:
                fillers.pop(0)()

        # ---- prologue: K^T(0) fully and Q^T(0) n=0, pipelined through the
        # (still idle) energies PSUM rotation. The scheduler is readiness-
        # driven with emission order as priority, so unit 0's energy fills
        # (emitted next) start the exp stream as soon as these finish; the
        # V tiles are emitted just below unit 0 and flow through the misc
        # bank and the (pre-attn@V) accumulator bank in parallel.
        emit_qk_ntile(wk_s, bkT_s, KT_sb, 0, 0, ps_e)
        emit_qk_ntile(wq_s, bqT_s, QT_sb, 0, 0, ps_e)

        prev = None
        for ui, u in enumerate(units):
            hp, j, s = u
            if j == 2 and s == 0 and hp < 3:
                # queue the next pair's Q/K tiles late enough that they
                # don't compete with the V burst at the head
                for w_s, b_s, dst in ((wq_s, bqT_s, QT_sb),
                                      (wk_s, bkT_s, KT_sb)):
                    for n in range(4):
                        fillers.append(
                            lambda w=w_s, b=b_s, d=dst, p=hp + 1, nn=n:
                            emit_qk_ntile(w, b, d, p, nn, ps_m))
            if s == 0:
                pair_ao = ao_pool.tile([128, 4, 128], bf16, tag="ao",
                                       name="pair_ao")
            else:
                pair_ao = state[prev][2]
            pba = pb_pool.tile([128, 8, 512], bf16, tag="pbA", name="pba",
                               bufs=4)
            pbb = pb_pool.tile([128, 8, 512], bf16, tag="pbB", name="pbb",
                               bufs=5)
            state[u] = ((pba, pbb), None, pair_ao)
            # energy fills + exp first (highest priority: they feed the
            # Activation stream); attn@V / normalization / fillers behind -
            # the readiness-driven scheduler slots them into PSUM-wait gaps.
            for g in range(8):
                emit_e_group(u, g)
                if ui == 0 and g in (1, 3, 5):
                    # K^T(0) tile n = g//2 + 1 just ahead of the energy
                    # groups that need it (group g reads K^T n-tile g//2)
                    emit_qk_ntile(wk_s, bkT_s, KT_sb, 0, g // 2 + 1, ps_e)
                if ui == 0 and g in (2, 4, 6):
                    # Q^T(0) n=1..3 likewise through the energies rotation
                    emit_qk_ntile(wq_s, bqT_s, QT_sb, 0, g // 2, ps_e)
            if ui == 0:
                # All V tiles must be EMITTED before attn@V(u0) (dependency
                # edges only link to previously-emitted writers), but their
                # scheduling priority is demoted so they fill the PE's
                # exp-wait bubbles instead of starving the energy fills.
                with tc.high_priority(offset=-150):
                    for t in range(16):
                        emit_v_tile(t, ps_m if t % 2 else ps_av)
            if prev is not None:
                # previous unit's attn@V / normalization: lower priority
                # than this unit's fills; the scheduler slots them into
                # exp-wait gaps.
                emit_av_block(prev, range(0, 8))
                emit_av_block(prev, range(8, 16))
                php, pj, ps_prev = prev
                pao = state[prev][2]
                emit_norm(prev)
                if ps_prev == 1:
                    emit_transposes(php, pj, pao)
                    if php == 3:
                        for tt in range(4):
                            fillers.append(
                                lambda t=4 * pj + tt: emit_proj(t))
            pop_filler()
            pop_filler()
            prev = u
        # pipeline tail
        emit_av_block(prev, range(0, 8))
        emit_av_block(prev, range(8, 16))
        pao = state[prev][2]
        emit_norm(prev)
        emit_transposes(3, 3, pao)
        for tt in range(4):
            fillers.append(lambda t=12 + tt: emit_proj(t, ps_e))
        while fillers:
            pop_filler()


def get_program():
    if "nc" not in _prog_cache:
        _prog_cache["nc"] = _build_program()
    return _prog_cache["nc"]


def make_in_maps(inputs):
    x = np.asarray(inputs["x"], dtype=np.float32)
    Wq = np.asarray(inputs["Wq"], dtype=np.float32)
    bq = np.asarray(inputs["bq"], dtype=np.float32)
    Wk = np.asarray(inputs["Wk"], dtype=np.float32)
    bk = np.asarray(inputs["bk"], dtype=np.float32)
    Wv = np.asarray(inputs["Wv"], dtype=np.float32)
    bv = np.asarray(inputs["bv"], dtype=np.float32)
    Wp = np.asarray(inputs["Wp"], dtype=np.float32)

    iden = np.eye(128, dtype=np.float32).astype(BF16)
    in_maps = []
    for c in range(N_CORES):
        b, half = divmod(c, 2)
        fs = slice(half * FH, half * FH + FH)
        in_maps.append({
            "xT": np.ascontiguousarray(x[b].T).astype(BF16),
            "wqT": np.ascontiguousarray(Wq[fs].T).astype(BF16),
            "wkT": np.ascontiguousarray(Wk[fs].T).astype(BF16),
            "wvT": np.ascontiguousarray(Wv[fs].T).astype(BF16),
            "bqkv": np.ascontiguousarray(np.concatenate(
                [bq[fs].reshape(4, 128).T, bk[fs].reshape(4, 128).T,
                 bv[fs].reshape(4, 128).T], axis=1)),
            "wpT": np.ascontiguousarray(Wp[:, fs].T).astype(BF16),
            "iden": iden,
        })
    return in_maps


def gather_output(results, bp):
    bp = np.asarray(bp, dtype=np.float32)
    return np.stack([
        results[2 * b]["out"].astype(np.float32)
        + results[2 * b + 1]["out"].astype(np.float32) + bp[None, :]
        for b in range(4)
    ]).astype(np.float32)


def kernel(**inputs):
    nc = get_program()
    in_maps = make_in_maps(inputs)
    res = run_bass_kernel_spmd(nc, in_maps, list(range(N_CORES))).results
    return gather_output(res, inputs["bp"])


# revision 3
# speedup vs baseline: 1.0010x; 1.0010x over previous
"""Multi-head attention (16 heads, d_model=1024, head_dim=64) on 8 trn2 cores.

Sharding: core c handles batch b = c//2 and heads [8*(c%2), 8*(c%2)+8)
(data parallel over batch x tensor parallel over heads). Each core
computes its 8 heads' Q/K/V projections, attention, and a partial output
projection; the host sums the two partial projections per batch element
(the "all-reduce") and adds the output bias bp.

Key layout choices (all driven by PE cost ~ moving-operand columns):
- Q^T/K^T are produced feature-major [d, t] so energies contract along
  partitions (stationary K^T chunk [64, 128], moving Q^T [64, 512]).
- attn@V is computed with the EXP WEIGHTS AS STATIONARY ([128 keys, 128
  queries]) and V as the moving operand ([128 keys, 65]): each matmul
  streams only 65 columns (64 head dims + a ones-column that yields the
  softmax row-sums), 8x fewer moving columns than the [d, t]-stationary
  orientation. The attention output lands QUERY-major [q, d], where the
  softmax normalization is a cheap per-partition scalar multiply.
- Normalized [q, f] tiles are transposed back to feature-major via PE
  transpose (128 cycles per 128x128 tile) to feed the output projection;
  the V bias is folded into the transpose-evacuation copy (rows of the
  normalized attention matrix sum to 1, so A@(V + bv) = A@V + bv).

Softmax is unnormalized exp (no max subtraction; energies bounded ~|15|)
with row sums taken by the ones-column of V. exp runs on the Activation
engine in 2-key-chunk calls over a 3-deep rotation of [128,2,512] PSUM
tiles - the deepest rotation that fits PSUM (6 banks + 1 attn@V
accumulator bank + 1 projection/transpose scratch bank) so the exp
stream decouples from the PE's fill cadence. The tile scheduler is
readiness-driven with emission order as priority: energy fills/exps are
emitted first each unit, attn@V / normalization / filler projections
behind them, and the 16 V-projection tiles are emitted inside unit 0
(dependency edges only link to previously-emitted writers) at demoted
priority so they soak up PE slack across units 1-4.
"""

import numpy as np
import ml_dtypes

from concourse import bass, bacc, tile, mybir
from concourse.bass_utils import run_bass_kernel_spmd

BF16 = ml_dtypes.bfloat16
dt = mybir.dt
AF = mybir.ActivationFunctionType

N_CORES = 8
T = 2048          # tokens per batch element
D = 1024          # model dim
FH = 512          # features (head dims) per core: 8 heads x 64
NH_LOC = 8        # heads per core
HD = 64           # head dim

_prog_cache = {}


def _build_program():
    nc = bacc.Bacc("TRN2", target_bir_lowering=False, debug=False,
                   num_devices=N_CORES)

    xT = nc.dram_tensor("xT", [D, T], dt.bfloat16, kind="ExternalInput").ap()
    wqT = nc.dram_tensor("wqT", [D, FH], dt.bfloat16, kind="ExternalInput").ap()
    wkT = nc.dram_tensor("wkT", [D, FH], dt.bfloat16, kind="ExternalInput").ap()
    wvT = nc.dram_tensor("wvT", [D, FH], dt.bfloat16, kind="ExternalInput").ap()
    bqkv = nc.dram_tensor("bqkv", [128, 12], dt.float32, kind="ExternalInput").ap()
    wpT = nc.dram_tensor("wpT", [FH, D], dt.bfloat16, kind="ExternalInput").ap()
    iden = nc.dram_tensor("iden", [128, 128], dt.bfloat16, kind="ExternalInput").ap()
    out = nc.dram_tensor("out", [T, D], dt.bfloat16, kind="ExternalOutput").ap()

    with tile.TileContext(nc) as tc:
        _emit(tc, out, xT, wqT, wkT, wvT, bqkv, wpT, iden)
    nc.compile()
    return nc


def _emit(tc, out, xT, wqT, wkT, wvT, bqkv, wpT, iden):
    nc = tc.nc
    f32 = dt.float32
    bf16 = dt.bfloat16

    with (
        tc.tile_pool(name="sbp", bufs=1) as sbp,
        tc.tile_pool(name="qkv_sb", bufs=1) as qkv_sb,
        tc.tile_pool(name="pb_pool", bufs=4) as pb_pool,
        tc.tile_pool(name="ao_pool", bufs=1) as ao_pool,
        tc.tile_pool(name="rr_pool", bufs=2) as rr_pool,
        tc.tile_pool(name="ostage", bufs=2) as ostage,
        # PSUM budget (8 banks): energies 3x[128,2,512] (6 banks) rotating
        # under the exp stream; attn@V accumulator bank (also used by half
        # the V-projection tiles at the head, before attn@V starts); one
        # shared bank for QK/V/proj chains and transpose evacuation.
        tc.tile_pool(name="ps_e", bufs=3, space="PSUM") as ps_e,
        tc.tile_pool(name="ps_av", bufs=1, space="PSUM") as ps_av,
        tc.tile_pool(name="ps_m", bufs=1, space="PSUM") as ps_m,
    ):
        # ---- input DMAs. The DMA engines share one per-core bandwidth
        # pool (~350 GB/s), so what matters is BYTE order, not queue count:
        # the K^T(0)/Q^T(0) n=0 tiles only touch tokens 0..511, so x is
        # loaded in token-quarters and the first exp can start after just
        # wk + x-quarter0 + wq (~3 MB) instead of all 8 MB of inputs.
        x_s = sbp.tile([128, 8, T], bf16)
        xr = xT.rearrange("(m p) t -> p m t", p=128)
        wk_s = sbp.tile([128, 8, FH], bf16, tag="wk")
        nc.sync.dma_start(out=wk_s[:], in_=wkT.rearrange("(m p) d -> p m d", p=128))
        nc.scalar.dma_start(out=x_s[:, :, 0:512], in_=xr[:, :, 0:512])
        wq_s = sbp.tile([128, 8, FH], bf16, tag="wq")
        nc.sync.dma_start(out=wq_s[:], in_=wqT.rearrange("(m p) d -> p m d", p=128))
        bqkv_s = sbp.tile([128, 12], f32)
        nc.scalar.dma_start(out=bqkv_s[:], in_=bqkv)
        bqT_s = bqkv_s[:, 0:4]
        bkT_s = bqkv_s[:, 4:8]
        bvT_s = bqkv_s[:, 8:12]
        nc.scalar.dma_start(out=x_s[:, :, 512:1024], in_=xr[:, :, 512:1024])
        nc.sync.dma_start(out=x_s[:, :, 1024:1536], in_=xr[:, :, 1024:1536])
        nc.scalar.dma_start(out=x_s[:, :, 1536:2048], in_=xr[:, :, 1536:2048])
        wv_s = sbp.tile([128, 8, FH], bf16, tag="wv")
        nc.sync.dma_start(out=wv_s[:], in_=wvT.rearrange("(m p) d -> p m d", p=128))
        id_s = sbp.tile([128, 128], bf16)
        nc.scalar.dma_start(out=id_s[:], in_=iden)
        wp_s = sbp.tile([128, 4, D], bf16)
        nc.scalar.dma_start(out=wp_s[:], in_=wpT.rearrange("(c p) o -> p c o", p=128))

        # QT/KT: [d-in-pair(128), head-pair(4), t]; V: [t-in-chunk(128),
        # t-chunk(16), head(8), 66] with col 64 = 1.0 (row-sum trick).
        QT_sb = qkv_sb.tile([128, 4, T], bf16)
        KT_sb = qkv_sb.tile([128, 4, T], bf16)
        V_sb = qkv_sb.tile([128, 16, NH_LOC, 65], bf16)
        nc.vector.memset(V_sb[:, :, :, 64:65], 1.0)
        # AttnOut^T (post-transpose): [f-in-chunk(128), f-chunk(4)=hp, t]
        AOT_sb = qkv_sb.tile([128, 4, T], bf16)

        def emit_qk_ntile(w_s, b_s, dst, hp, n, pool):
            # one n-tile of a Q^T/K^T projection: an 8-matmul chain + bias
            dsl = slice(hp * 128, (hp + 1) * 128)
            ps = pool.tile([128, 512], f32, tag="e" if pool is ps_e else "m",
                           name="qk_ps")
            for m in range(8):
                nc.tensor.matmul(ps[:], w_s[:, m, dsl],
                                 x_s[:, m, n * 512:(n + 1) * 512],
                                 start=(m == 0), stop=(m == 7))
            nc.vector.tensor_scalar_add(
                dst[:, hp, n * 512:(n + 1) * 512], ps[:], b_s[:, hp:hp + 1])

        def emit_v_tile(t, pool):
            # V (natural): out[t, f] = x[t, :].wvT[:, f]  (bias folded later)
            tag = {id(ps_e): "e", id(ps_av): "av", id(ps_m): "m"}[id(pool)]
            ps = pool.tile([128, 512], f32, tag=tag, name="v_ps")
            for m in range(8):
                nc.tensor.matmul(ps[:], x_s[:, m, t * 128:(t + 1) * 128],
                                 wv_s[:, m, :], start=(m == 0), stop=(m == 7))
            nc.vector.tensor_copy(
                V_sb[:, t, :, 0:64],
                ps[:].rearrange("p (h d) -> p h d", h=NH_LOC))

        def emit_proj(t, pool=None):
            # partial output projection (pre-bias) for token tile t
            pool = pool or ps_m
            tag = {id(ps_e): "e", id(ps_av): "av", id(ps_m): "m"}[id(pool)]
            tsl = slice(t * 128, (t + 1) * 128)
            st = ostage.tile([128, D], bf16, tag="st")
            for half in range(2):
                ps = pool.tile([128, 512], f32, tag=tag, name="pj")
                for fc in range(4):
                    nc.tensor.matmul(ps[:], AOT_sb[:, fc, tsl],
                                     wp_s[:, fc, half * 512:(half + 1) * 512],
                                     start=(fc == 0), stop=(fc == 3))
                nc.vector.tensor_copy(st[:, half * 512:(half + 1) * 512], ps[:])
            nc.sync.dma_start(out=out[tsl, :], in_=st[:])

        # ---- software-pipelined attention ----
        units = [(hp, j, s) for hp in range(4) for j in range(4)
                 for s in range(2)]
        state = {}      # u -> (pb, av) live tiles
        fillers = []    # queue of zero-arg emitters

        GROUPS = [(2 * g, 2 * g + 2) for g in range(8)]

        def emit_e_group(u, g):
            # fill one energies PSUM tile (3 key-chunks, 1 for the last
            # group) and exp it into pb
            hp, j, s = u
            psl = slice(64 * s, 64 * s + 64)
            qsl = slice(j * 512, (j + 1) * 512)
            pb = state[u][0][0] if g < 4 else state[u][0][1]
            lo, hi = GROUPS[g]
            off = 0 if g < 4 else 8
            w = hi - lo
            et = ps_e.tile([128, 2, 512], f32, tag="e", name="et")
            for i in range(w):
                kc = lo + i
                nc.tensor.matmul(et[:, i, :], KT_sb[psl, hp, kc * 128:(kc + 1) * 128],
                                 QT_sb[psl, hp, qsl], start=True, stop=True)
            nc.scalar.activation(pb[:, lo - off:hi - off, :], et[:, 0:w, :], AF.Exp)

        def emit_av_block(u, kcs, pool=None):
            # attn@V with pb as stationary: out[q, d] accumulates per
            # q-chunk; V col 64 (ones) accumulates the softmax row sums.
            hp, j, s = u
            h = 2 * hp + s
            (pba, pbb), av = state[u][0], state[u][1]
            if av is None:
                pool = pool or ps_av
                tag = {id(ps_e): "e", id(ps_av): "av", id(ps_m): "m"}[id(pool)]
                avt = pool.tile([128, 512], f32, tag=tag, name="avt")
                av = avt[:, 0:260].rearrange("p (q c) -> p q c", c=65)
                state[u] = ((pba, pbb), av, state[u][2])
            # One PSUM accumulation group for the WHOLE tile: start=True
            # zeroes the entire 2KB zero-region, so only the first matmul
            # emitted may carry it, and only the last carries stop (the
            # framework serializes accumulates into the same tile, so
            # emission order is execution order here).
            for qc in range(4):
                for kc in kcs:
                    pb = pba if kc < 8 else pbb
                    nc.tensor.matmul(
                        av[:, qc, :],
                        pb[:, kc % 8, qc * 128:(qc + 1) * 128],
                        V_sb[:, kc, h, 0:65],
                        start=(qc == 0 and kc == 0),
                        stop=(qc == 3 and kc == 15))

        def emit_norm(u, with_tr=False, with_proj=False):
            # softmax normalization: per-partition (per-query) reciprocal
            # of the row sums, then scale the 64 head dims into AO staging
            # [q, qsub, pair-features]. For s==1 units the per-q-chunk
            # transpose back to feature-major (+ V-bias add) is fused in so
            # the chain pipelines per chunk; for the final block each
            # chunk's projection tile follows immediately.
            hp, j, s = u
            av = state[u][1]
            ao = state[u][2]
            rr = rr_pool.tile([128, 4], f32, tag="rr", bufs=4)
            nc.vector.reciprocal(rr[:], av[:, :, 64])
            for qc in range(4):
                nc.vector.tensor_scalar_mul(
                    ao[:, qc, s * 64:(s + 1) * 64], av[:, qc, 0:64],
                    rr[:, qc:qc + 1])
                if with_tr:
                    tp = ps_m.tile([128, 128], bf16, tag="m", name="tp")
                    nc.tensor.transpose(tp[:], ao[:, qc, :], id_s[:])
                    nc.vector.tensor_scalar_add(
                        AOT_sb[:, hp,
                               j * 512 + qc * 128:j * 512 + (qc + 1) * 128],
                        tp[:], bvT_s[:, hp:hp + 1])
                    if with_proj:
                        emit_proj(4 * j + qc, ps_e)
            del state[u]

        def pop_filler():
            if fillers:
                fillers.pop(0)()

        # ---- prologue: K^T(0) fully and Q^T(0) n=0, pipelined through the
        # (still idle) energies PSUM rotation. The scheduler is readiness-
        # driven with emission order as priority, so unit 0's energy fills
        # (emitted next) start the exp stream as soon as these finish; the
        # V tiles are emitted just below unit 0 and flow through the misc
        # bank and the (pre-attn@V) accumulator bank in parallel.
        emit_qk_ntile(wk_s, bkT_s, KT_sb, 0, 0, ps_e)
        emit_qk_ntile(wq_s, bqT_s, QT_sb, 0, 0, ps_e)

        prev = None
        for ui, u in enumerate(units):
            hp, j, s = u
            if j == 2 and s == 0 and hp < 3:
                # queue the next pair's Q/K tiles late enough that they
                # don't compete with the V burst at the head
                for w_s, b_s, dst in ((wq_s, bqT_s, QT_sb),
                                      (wk_s, bkT_s, KT_sb)):
                    for n in range(4):
                        fillers.append(
                            lambda w=w_s, b=b_s, d=dst, p=hp + 1, nn=n:
                            emit_qk_ntile(w, b, d, p, nn, ps_m))
            if s == 0:
                pair_ao = ao_pool.tile([128, 4, 128], bf16, tag="ao",
                                       name="pair_ao")
            else:
                pair_ao = state[prev][2]
            pba = pb_pool.tile([128, 8, 512], bf16, tag="pbA", name="pba",
                               bufs=4)
            pbb = pb_pool.tile([128, 8, 512], bf16, tag="pbB", name="pbb",
                               bufs=5)
            state[u] = ((pba, pbb), None, pair_ao)
            # energy fills + exp first (highest priority: they feed the
            # Activation stream); attn@V / normalization / fillers behind -
            # the readiness-driven scheduler slots them into PSUM-wait gaps.
            for g in range(8):
                emit_e_group(u, g)
                if ui == 0 and g in (1, 3, 5):
                    # K^T(0) tile n = g//2 + 1 just ahead of the energy
                    # groups that need it (group g reads K^T n-tile g//2)
                    emit_qk_ntile(wk_s, bkT_s, KT_sb, 0, g // 2 + 1, ps_e)
                if ui == 0 and g in (2, 4, 6):
                    # Q^T(0) n=1..3 likewise through the energies rotation
                    emit_qk_ntile(wq_s, bqT_s, QT_sb, 0, g // 2, ps_e)
            if ui == 0:
                # All V tiles must be EMITTED before attn@V(u0) (dependency
                # edges only link to previously-emitted writers), but their
                # scheduling priority is demoted so they fill the PE's
                # exp-wait bubbles instead of starving the energy fills.
                with tc.high_priority(offset=-150):
                    for t in range(16):
                        emit_v_tile(t, ps_m if t % 2 else ps_av)
            if prev is not None:
                # previous unit's attn@V / normalization: lower priority
                # than this unit's fills; the scheduler slots them into
                # exp-wait gaps.
                emit_av_block(prev, range(0, 8))
                emit_av_block(prev, range(8, 16))
                php, pj, ps_prev = prev
                emit_norm(prev, with_tr=(ps_prev == 1))
                if ps_prev == 1 and php == 3 and pj < 3:
                    for tt in range(4):
                        fillers.append(
                            lambda t=4 * pj + tt: emit_proj(t))
            pop_filler()
            pop_filler()
            prev = u
        # pipeline tail: last unit's attn@V through the (now idle)
        # energies rotation so it overlaps the final exps, then per-chunk
        # norm -> transpose -> projection
        emit_av_block(prev, range(0, 8), pool=ps_e)
        emit_av_block(prev, range(8, 16))
        emit_norm(prev, with_tr=True, with_proj=True)
        while fillers:
            pop_filler()


def get_program():
    if "nc" not in _prog_cache:
        _prog_cache["nc"] = _build_program()
    return _prog_cache["nc"]


def make_in_maps(inputs):
    x = np.asarray(inputs["x"], dtype=np.float32)
    Wq = np.asarray(inputs["Wq"], dtype=np.float32)
    bq = np.asarray(inputs["bq"], dtype=np.float32)
    Wk = np.asarray(inputs["Wk"], dtype=np.float32)
    bk = np.asarray(inputs["bk"], dtype=np.float32)
    Wv = np.asarray(inputs["Wv"], dtype=np.float32)
    bv = np.asarray(inputs["bv"], dtype=np.float32)
    Wp = np.asarray(inputs["Wp"], dtype=np.float32)

    iden = np.eye(128, dtype=np.float32).astype(BF16)
    in_maps = []
    for c in range(N_CORES):
        b, half = divmod(c, 2)
        fs = slice(half * FH, half * FH + FH)
        in_maps.append({
            "xT": np.ascontiguousarray(x[b].T).astype(BF16),
            "wqT": np.ascontiguousarray(Wq[fs].T).astype(BF16),
            "wkT": np.ascontiguousarray(Wk[fs].T).astype(BF16),
            "wvT": np.ascontiguousarray(Wv[fs].T).astype(BF16),
            "bqkv": np.ascontiguousarray(np.concatenate(
                [bq[fs].reshape(4, 128).T, bk[fs].reshape(4, 128).T,
                 bv[fs].reshape(4, 128).T], axis=1)),
            "wpT": np.ascontiguousarray(Wp[:, fs].T).astype(BF16),
            "iden": iden,
        })
    return in_maps


def gather_output(results, bp):
    bp = np.asarray(bp, dtype=np.float32)
    return np.stack([
        results[2 * b]["out"].astype(np.float32)
        + results[2 * b + 1]["out"].astype(np.float32) + bp[None, :]
        for b in range(4)
    ]).astype(np.float32)


def kernel(**inputs):
    nc = get_program()
    in_maps = make_in_maps(inputs)
    res = run_bass_kernel_spmd(nc, in_maps, list(range(N_CORES))).results
    return gather_output(res, inputs["bp"])


# revision 4
# speedup vs baseline: 1.0046x; 1.0036x over previous
"""Multi-head attention (16 heads, d_model=1024, head_dim=64) on 8 trn2 cores.

Sharding: core c handles batch b = c//2 and heads [8*(c%2), 8*(c%2)+8)
(data parallel over batch x tensor parallel over heads). Each core
computes its 8 heads' Q/K/V projections, attention, and a partial output
projection; the host sums the two partial projections per batch element
(the "all-reduce") and adds the output bias bp.

Key layout choices (all driven by PE cost ~ moving-operand columns):
- Q^T/K^T are produced feature-major [d, t] so energies contract along
  partitions (stationary K^T chunk [64, 128], moving Q^T [64, 512]).
- attn@V is computed with the EXP WEIGHTS AS STATIONARY ([128 keys, 128
  queries]) and V as the moving operand ([128 keys, 65]): each matmul
  streams only 65 columns (64 head dims + a ones-column that yields the
  softmax row-sums), 8x fewer moving columns than the [d, t]-stationary
  orientation. The attention output lands QUERY-major [q, d], where the
  softmax normalization is a cheap per-partition scalar multiply.
- Normalized [q, f] tiles are transposed back to feature-major via PE
  transpose (128 cycles per 128x128 tile) to feed the output projection;
  the V bias is folded into the transpose-evacuation copy (rows of the
  normalized attention matrix sum to 1, so A@(V + bv) = A@V + bv).

Softmax is unnormalized exp (no max subtraction; energies bounded ~|15|)
with row sums taken by the ones-column of V. exp runs on the Activation
engine in 2-key-chunk calls over a 3-deep rotation of [128,2,512] PSUM
tiles - the deepest rotation that fits PSUM (6 banks + 1 attn@V
accumulator bank + 1 projection/transpose scratch bank) so the exp
stream decouples from the PE's fill cadence. The tile scheduler is
readiness-driven with emission order as priority: energy fills/exps are
emitted first each unit, attn@V / normalization / filler projections
behind them, and the 16 V-projection tiles are emitted inside unit 0
(dependency edges only link to previously-emitted writers) at demoted
priority so they soak up PE slack across units 1-4.
"""

import numpy as np
import ml_dtypes

from concourse import bass, bacc, tile, mybir
from concourse.bass_utils import run_bass_kernel_spmd

BF16 = ml_dtypes.bfloat16
dt = mybir.dt
AF = mybir.ActivationFunctionType

N_CORES = 8
T = 2048          # tokens per batch element
D = 1024          # model dim
FH = 512          # features (head dims) per core: 8 heads x 64
NH_LOC = 8        # heads per core
HD = 64           # head dim

_prog_cache = {}


def _build_program():
    nc = bacc.Bacc("TRN2", target_bir_lowering=False, debug=False,
                   num_devices=N_CORES)

    xT = nc.dram_tensor("xT", [D, T], dt.bfloat16, kind="ExternalInput").ap()
    wqT = nc.dram_tensor("wqT", [D, FH], dt.bfloat16, kind="ExternalInput").ap()
    wkT = nc.dram_tensor("wkT", [D, FH], dt.bfloat16, kind="ExternalInput").ap()
    wvT = nc.dram_tensor("wvT", [D, FH], dt.bfloat16, kind="ExternalInput").ap()
    bqkv = nc.dram_tensor("bqkv", [128, 12], dt.float32, kind="ExternalInput").ap()
    wpT = nc.dram_tensor("wpT", [FH, D], dt.bfloat16, kind="ExternalInput").ap()
    iden = nc.dram_tensor("iden", [128, 128], dt.bfloat16, kind="ExternalInput").ap()
    out = nc.dram_tensor("out", [T, D], dt.bfloat16, kind="ExternalOutput").ap()

    with tile.TileContext(nc) as tc:
        _emit(tc, out, xT, wqT, wkT, wvT, bqkv, wpT, iden)
    nc.compile()
    return nc


def _emit(tc, out, xT, wqT, wkT, wvT, bqkv, wpT, iden):
    nc = tc.nc
    f32 = dt.float32
    bf16 = dt.bfloat16

    with (
        tc.tile_pool(name="sbp", bufs=1) as sbp,
        tc.tile_pool(name="qkv_sb", bufs=1) as qkv_sb,
        tc.tile_pool(name="pb_pool", bufs=4) as pb_pool,
        tc.tile_pool(name="ao_pool", bufs=1) as ao_pool,
        tc.tile_pool(name="rr_pool", bufs=2) as rr_pool,
        tc.tile_pool(name="ostage", bufs=2) as ostage,
        # PSUM budget (8 banks): energies 3x[128,2,512] (6 banks) rotating
        # under the exp stream; attn@V accumulator bank (also used by half
        # the V-projection tiles at the head, before attn@V starts); one
        # shared bank for QK/V/proj chains and transpose evacuation.
        tc.tile_pool(name="ps_e", bufs=3, space="PSUM") as ps_e,
        tc.tile_pool(name="ps_av", bufs=1, space="PSUM") as ps_av,
        tc.tile_pool(name="ps_m", bufs=1, space="PSUM") as ps_m,
    ):
        # ---- input DMAs. The DMA engines share one per-core bandwidth
        # pool (~350 GB/s), so what matters is BYTE order, not queue count:
        # the K^T(0)/Q^T(0) n=0 tiles only touch tokens 0..511, so x is
        # loaded in token-quarters and the first exp can start after just
        # wk + x-quarter0 + wq (~3 MB) instead of all 8 MB of inputs.
        x_s = sbp.tile([128, 8, T], bf16)
        xr = xT.rearrange("(m p) t -> p m t", p=128)
        wk_s = sbp.tile([128, 8, FH], bf16, tag="wk")
        nc.sync.dma_start(out=wk_s[:], in_=wkT.rearrange("(m p) d -> p m d", p=128))
        nc.scalar.dma_start(out=x_s[:, :, 0:512], in_=xr[:, :, 0:512])
        wq_s = sbp.tile([128, 8, FH], bf16, tag="wq")
        nc.sync.dma_start(out=wq_s[:], in_=wqT.rearrange("(m p) d -> p m d", p=128))
        bqkv_s = sbp.tile([128, 12], f32)
        nc.scalar.dma_start(out=bqkv_s[:], in_=bqkv)
        bqT_s = bqkv_s[:, 0:4]
        bkT_s = bqkv_s[:, 4:8]
        bvT_s = bqkv_s[:, 8:12]
        nc.scalar.dma_start(out=x_s[:, :, 512:1024], in_=xr[:, :, 512:1024])
        wv_s = sbp.tile([128, 8, FH], bf16, tag="wv")
        nc.sync.dma_start(out=wv_s[:], in_=wvT.rearrange("(m p) d -> p m d", p=128))
        nc.sync.dma_start(out=x_s[:, :, 1024:1536], in_=xr[:, :, 1024:1536])
        nc.scalar.dma_start(out=x_s[:, :, 1536:2048], in_=xr[:, :, 1536:2048])
        id_s = sbp.tile([128, 128], bf16)
        nc.scalar.dma_start(out=id_s[:], in_=iden)
        wp_s = sbp.tile([128, 4, D], bf16)
        nc.scalar.dma_start(out=wp_s[:], in_=wpT.rearrange("(c p) o -> p c o", p=128))

        # QT/KT: [d-in-pair(128), head-pair(4), t]; V: [t-in-chunk(128),
        # t-chunk(16), head(8), 66] with col 64 = 1.0 (row-sum trick).
        QT_sb = qkv_sb.tile([128, 4, T], bf16)
        KT_sb = qkv_sb.tile([128, 4, T], bf16)
        V_sb = qkv_sb.tile([128, 16, NH_LOC, 65], bf16)
        nc.vector.memset(V_sb[:, :, :, 64:65], 1.0)
        # AttnOut^T (post-transpose): [f-in-chunk(128), f-chunk(4)=hp, t]
        AOT_sb = qkv_sb.tile([128, 4, T], bf16)

        def emit_qk_ntile(w_s, b_s, dst, hp, n, pool):
            # one n-tile of a Q^T/K^T projection: an 8-matmul chain + bias
            dsl = slice(hp * 128, (hp + 1) * 128)
            ps = pool.tile([128, 512], f32, tag="e" if pool is ps_e else "m",
                           name="qk_ps")
            for m in range(8):
                nc.tensor.matmul(ps[:], w_s[:, m, dsl],
                                 x_s[:, m, n * 512:(n + 1) * 512],
                                 start=(m == 0), stop=(m == 7))
            nc.vector.tensor_scalar_add(
                dst[:, hp, n * 512:(n + 1) * 512], ps[:], b_s[:, hp:hp + 1])

        def emit_v_tile(t, pool):
            # V (natural): out[t, f] = x[t, :].wvT[:, f]  (bias folded later)
            tag = {id(ps_e): "e", id(ps_av): "av", id(ps_m): "m"}[id(pool)]
            ps = pool.tile([128, 512], f32, tag=tag, name="v_ps")
            for m in range(8):
                nc.tensor.matmul(ps[:], x_s[:, m, t * 128:(t + 1) * 128],
                                 wv_s[:, m, :], start=(m == 0), stop=(m == 7))
            nc.vector.tensor_copy(
                V_sb[:, t, :, 0:64],
                ps[:].rearrange("p (h d) -> p h d", h=NH_LOC))

        def emit_proj(t, pool=None):
            # partial output projection (pre-bias) for token tile t
            pool = pool or ps_m
            tag = {id(ps_e): "e", id(ps_av): "av", id(ps_m): "m"}[id(pool)]
            tsl = slice(t * 128, (t + 1) * 128)
            st = ostage.tile([128, D], bf16, tag="st")
            for half in range(2):
                ps = pool.tile([128, 512], f32, tag=tag, name="pj")
                for fc in range(4):
                    nc.tensor.matmul(ps[:], AOT_sb[:, fc, tsl],
                                     wp_s[:, fc, half * 512:(half + 1) * 512],
                                     start=(fc == 0), stop=(fc == 3))
                nc.vector.tensor_copy(st[:, half * 512:(half + 1) * 512], ps[:])
            nc.sync.dma_start(out=out[tsl, :], in_=st[:])

        # ---- software-pipelined attention ----
        units = [(hp, j, s) for hp in range(4) for j in range(4)
                 for s in range(2)]
        state = {}      # u -> (pb, av) live tiles
        fillers = []    # queue of zero-arg emitters

        GROUPS = [(2 * g, 2 * g + 2) for g in range(8)]

        def emit_e_group(u, g):
            # fill one energies PSUM tile (3 key-chunks, 1 for the last
            # group) and exp it into pb
            hp, j, s = u
            psl = slice(64 * s, 64 * s + 64)
            qsl = slice(j * 512, (j + 1) * 512)
            pb = state[u][0][0] if g < 4 else state[u][0][1]
            lo, hi = GROUPS[g]
            off = 0 if g < 4 else 8
            w = hi - lo
            et = ps_e.tile([128, 2, 512], f32, tag="e", name="et")
            for i in range(w):
                kc = lo + i
                nc.tensor.matmul(et[:, i, :], KT_sb[psl, hp, kc * 128:(kc + 1) * 128],
                                 QT_sb[psl, hp, qsl], start=True, stop=True)
            nc.scalar.activation(pb[:, lo - off:hi - off, :], et[:, 0:w, :], AF.Exp)

        def emit_av_block(u, kcs, pool=None):
            # attn@V with pb as stationary: out[q, d] accumulates per
            # q-chunk; V col 64 (ones) accumulates the softmax row sums.
            hp, j, s = u
            h = 2 * hp + s
            (pba, pbb), av = state[u][0], state[u][1]
            if av is None:
                pool = pool or ps_av
                tag = {id(ps_e): "e", id(ps_av): "av", id(ps_m): "m"}[id(pool)]
                avt = pool.tile([128, 512], f32, tag=tag, name="avt")
                av = avt[:, 0:260].rearrange("p (q c) -> p q c", c=65)
                state[u] = ((pba, pbb), av, state[u][2])
            # One PSUM accumulation group for the WHOLE tile: start=True
            # zeroes the entire 2KB zero-region, so only the first matmul
            # emitted may carry it, and only the last carries stop (the
            # framework serializes accumulates into the same tile, so
            # emission order is execution order here).
            for qc in range(4):
                for kc in kcs:
                    pb = pba if kc < 8 else pbb
                    nc.tensor.matmul(
                        av[:, qc, :],
                        pb[:, kc % 8, qc * 128:(qc + 1) * 128],
                        V_sb[:, kc, h, 0:65],
                        start=(qc == 0 and kc == 0),
                        stop=(qc == 3 and kc == 15))

        def emit_norm(u, with_tr=False, with_proj=False):
            # softmax normalization: per-partition (per-query) reciprocal
            # of the row sums, then scale the 64 head dims into AO staging
            # [q, qsub, pair-features]. For s==1 units the per-q-chunk
            # transpose back to feature-major (+ V-bias add) is fused in so
            # the chain pipelines per chunk; for the final block each
            # chunk's projection tile follows immediately.
            hp, j, s = u
            av = state[u][1]
            ao = state[u][2]
            rr = rr_pool.tile([128, 4], f32, tag="rr", bufs=4)
            nc.vector.reciprocal(rr[:], av[:, :, 64])
            for qc in range(4):
                nc.vector.tensor_scalar_mul(
                    ao[:, qc, s * 64:(s + 1) * 64], av[:, qc, 0:64],
                    rr[:, qc:qc + 1])
                if with_tr:
                    tp = ps_m.tile([128, 128], bf16, tag="m", name="tp")
                    nc.tensor.transpose(tp[:], ao[:, qc, :], id_s[:])
                    nc.vector.tensor_scalar_add(
                        AOT_sb[:, hp,
                               j * 512 + qc * 128:j * 512 + (qc + 1) * 128],
                        tp[:], bvT_s[:, hp:hp + 1])
                    if with_proj:
                        emit_proj(4 * j + qc, ps_e)
            del state[u]

        def pop_filler():
            if fillers:
                fillers.pop(0)()

        # ---- prologue: K^T(0) fully and Q^T(0) n=0, pipelined through the
        # (still idle) energies PSUM rotation. The scheduler is readiness-
        # driven with emission order as priority, so unit 0's energy fills
        # (emitted next) start the exp stream as soon as these finish; the
        # V tiles are emitted just below unit 0 and flow through the misc
        # bank and the (pre-attn@V) accumulator bank in parallel.
        emit_qk_ntile(wk_s, bkT_s, KT_sb, 0, 0, ps_e)
        emit_qk_ntile(wq_s, bqT_s, QT_sb, 0, 0, ps_e)

        prev = None
        for ui, u in enumerate(units):
            hp, j, s = u
            if j == 2 and s == 0 and hp < 3:
                # queue the next pair's Q/K tiles late enough that they
                # don't compete with the V burst at the head
                for w_s, b_s, dst in ((wq_s, bqT_s, QT_sb),
                                      (wk_s, bkT_s, KT_sb)):
                    for n in range(4):
                        fillers.append(
                            lambda w=w_s, b=b_s, d=dst, p=hp + 1, nn=n:
                            emit_qk_ntile(w, b, d, p, nn, ps_m))
            if s == 0:
                pair_ao = ao_pool.tile([128, 4, 128], bf16, tag="ao",
                                       name="pair_ao")
            else:
                pair_ao = state[prev][2]
            pba = pb_pool.tile([128, 8, 512], bf16, tag="pbA", name="pba",
                               bufs=4)
            pbb = pb_pool.tile([128, 8, 512], bf16, tag="pbB", name="pbb",
                               bufs=5)
            state[u] = ((pba, pbb), None, pair_ao)
            # energy fills + exp first (highest priority: they feed the
            # Activation stream); attn@V / normalization / fillers behind -
            # the readiness-driven scheduler slots them into PSUM-wait gaps.
            for g in range(8):
                emit_e_group(u, g)
                if ui == 0 and g in (1, 3, 5):
                    # K^T(0) tile n = g//2 + 1 just ahead of the energy
                    # groups that need it (group g reads K^T n-tile g//2)
                    emit_qk_ntile(wk_s, bkT_s, KT_sb, 0, g // 2 + 1, ps_e)
                if ui == 0 and g in (2, 4, 6):
                    # Q^T(0) n=1..3 likewise through the energies rotation
                    emit_qk_ntile(wq_s, bqT_s, QT_sb, 0, g // 2, ps_e)
            if ui == 0:
                # All V tiles must be EMITTED before attn@V(u0) (dependency
                # edges only link to previously-emitted writers), but their
                # scheduling priority is demoted so they fill the PE's
                # exp-wait bubbles instead of starving the energy fills.
                with tc.high_priority(offset=-150):
                    for t in range(16):
                        emit_v_tile(t, ps_m if t % 2 else ps_av)
            if prev is not None:
                # previous unit's attn@V / normalization: lower priority
                # than this unit's fills; the scheduler slots them into
                # exp-wait gaps. Alternating the accumulator between the
                # attn@V bank and the misc bank decouples consecutive
                # units' av chains from each other's normalization reads.
                emit_av_block(prev, range(0, 8))
                emit_av_block(prev, range(8, 16))
                php, pj, ps_prev = prev
                emit_norm(prev, with_tr=(ps_prev == 1))
                if ps_prev == 1 and php == 3 and pj < 3:
                    for tt in range(4):
                        fillers.append(
                            lambda t=4 * pj + tt: emit_proj(t))
            pop_filler()
            pop_filler()
            prev = u
        # pipeline tail: last unit's attn@V through the (now idle)
        # energies rotation so it overlaps the final exps, then per-chunk
        # norm -> transpose -> projection
        emit_av_block(prev, range(0, 8), pool=ps_e)
        emit_av_block(prev, range(8, 16))
        emit_norm(prev, with_tr=True, with_proj=True)
        while fillers:
            pop_filler()


def get_program():
    if "nc" not in _prog_cache:
        _prog_cache["nc"] = _build_program()
    return _prog_cache["nc"]


def make_in_maps(inputs):
    x = np.asarray(inputs["x"], dtype=np.float32)
    Wq = np.asarray(inputs["Wq"], dtype=np.float32)
    bq = np.asarray(inputs["bq"], dtype=np.float32)
    Wk = np.asarray(inputs["Wk"], dtype=np.float32)
    bk = np.asarray(inputs["bk"], dtype=np.float32)
    Wv = np.asarray(inputs["Wv"], dtype=np.float32)
    bv = np.asarray(inputs["bv"], dtype=np.float32)
    Wp = np.asarray(inputs["Wp"], dtype=np.float32)

    iden = np.eye(128, dtype=np.float32).astype(BF16)
    in_maps = []
    for c in range(N_CORES):
        b, half = divmod(c, 2)
        fs = slice(half * FH, half * FH + FH)
        in_maps.append({
            "xT": np.ascontiguousarray(x[b].T).astype(BF16),
            "wqT": np.ascontiguousarray(Wq[fs].T).astype(BF16),
            "wkT": np.ascontiguousarray(Wk[fs].T).astype(BF16),
            "wvT": np.ascontiguousarray(Wv[fs].T).astype(BF16),
            "bqkv": np.ascontiguousarray(np.concatenate(
                [bq[fs].reshape(4, 128).T, bk[fs].reshape(4, 128).T,
                 bv[fs].reshape(4, 128).T], axis=1)),
            "wpT": np.ascontiguousarray(Wp[:, fs].T).astype(BF16),
            "iden": iden,
        })
    return in_maps


def gather_output(results, bp):
    bp = np.asarray(bp, dtype=np.float32)
    return np.stack([
        results[2 * b]["out"].astype(np.float32)
        + results[2 * b + 1]["out"].astype(np.float32) + bp[None, :]
        for b in range(4)
    ]).astype(np.float32)


def kernel(**inputs):
    nc = get_program()
    in_maps = make_in_maps(inputs)
    res = run_bass_kernel_spmd(nc, in_maps, list(range(N_CORES))).results
    return gather_output(res, inputs["bp"])


# revision 5
# speedup vs baseline: 1.0150x; 1.0103x over previous
"""Multi-head attention (16 heads, d_model=1024, head_dim=64) on 8 trn2 cores.

Sharding: core c handles batch b = c//2 and heads [8*(c%2), 8*(c%2)+8)
(data parallel over batch x tensor parallel over heads). Each core
computes its 8 heads' Q/K/V projections, attention, and a partial output
projection; the host sums the two partial projections per batch element
(the "all-reduce") and adds the output bias bp.

Key layout choices (all driven by PE cost ~ moving-operand columns):
- Q^T/K^T are produced feature-major [d, t] so energies contract along
  partitions (stationary K^T chunk [64, 128], moving Q^T [64, 512]).
- attn@V is computed with the EXP WEIGHTS AS STATIONARY ([128 keys, 128
  queries]) and V as the moving operand ([128 keys, 65]): each matmul
  streams only 65 columns (64 head dims + a ones-column that yields the
  softmax row-sums), 8x fewer moving columns than the [d, t]-stationary
  orientation. The attention output lands QUERY-major [q, d], where the
  softmax normalization is a cheap per-partition scalar multiply.
- Normalized [q, f] tiles are transposed back to feature-major via PE
  transpose (128 cycles per 128x128 tile) to feed the output projection;
  the V bias is folded into the transpose-evacuation copy (rows of the
  normalized attention matrix sum to 1, so A@(V + bv) = A@V + bv).

Softmax is unnormalized exp (no max subtraction; energies bounded ~|15|)
with row sums taken by the ones-column of V. exp runs on the Activation
engine in 2-key-chunk calls over a 3-deep rotation of [128,2,512] PSUM
tiles - the deepest rotation that fits PSUM (6 banks + 1 attn@V
accumulator bank + 1 projection/transpose scratch bank) so the exp
stream decouples from the PE's fill cadence. The tile scheduler is
readiness-driven with emission order as priority: energy fills/exps are
emitted first each unit, attn@V / normalization / filler projections
behind them, and the 16 V-projection tiles are emitted inside unit 0
(dependency edges only link to previously-emitted writers) at demoted
priority so they soak up PE slack across units 1-4.
"""

import numpy as np
import ml_dtypes

from concourse import bass, bacc, tile, mybir
from concourse.bass_utils import run_bass_kernel_spmd

BF16 = ml_dtypes.bfloat16
dt = mybir.dt
AF = mybir.ActivationFunctionType

N_CORES = 8
T = 2048          # tokens per batch element
D = 1024          # model dim
FH = 512          # features (head dims) per core: 8 heads x 64
NH_LOC = 8        # heads per core
HD = 64           # head dim

_prog_cache = {}


def _build_program():
    nc = bacc.Bacc("TRN2", target_bir_lowering=False, debug=False,
                   num_devices=N_CORES)

    xT = nc.dram_tensor("xT", [D, T], dt.bfloat16, kind="ExternalInput").ap()
    wqT = nc.dram_tensor("wqT", [D, FH], dt.bfloat16, kind="ExternalInput").ap()
    wkT = nc.dram_tensor("wkT", [D, FH], dt.bfloat16, kind="ExternalInput").ap()
    wvT = nc.dram_tensor("wvT", [D, FH], dt.bfloat16, kind="ExternalInput").ap()
    bqkv = nc.dram_tensor("bqkv", [128, 12], dt.float32, kind="ExternalInput").ap()
    wpT = nc.dram_tensor("wpT", [FH, D], dt.bfloat16, kind="ExternalInput").ap()
    iden = nc.dram_tensor("iden", [128, 128], dt.bfloat16, kind="ExternalInput").ap()
    out = nc.dram_tensor("out", [T, D], dt.bfloat16, kind="ExternalOutput").ap()

    with tile.TileContext(nc) as tc:
        _emit(tc, out, xT, wqT, wkT, wvT, bqkv, wpT, iden)
    nc.compile()
    return nc


def _emit(tc, out, xT, wqT, wkT, wvT, bqkv, wpT, iden):
    nc = tc.nc
    f32 = dt.float32
    bf16 = dt.bfloat16

    with (
        tc.tile_pool(name="sbp", bufs=1) as sbp,
        tc.tile_pool(name="qkv_sb", bufs=1) as qkv_sb,
        tc.tile_pool(name="pb_pool", bufs=4) as pb_pool,
        tc.tile_pool(name="ao_pool", bufs=1) as ao_pool,
        tc.tile_pool(name="rr_pool", bufs=2) as rr_pool,
        tc.tile_pool(name="ostage", bufs=2) as ostage,
        # PSUM budget (8 banks): energies 3x[128,2,512] (6 banks) rotating
        # under the exp stream; attn@V accumulator bank (also used by half
        # the V-projection tiles at the head, before attn@V starts); one
        # shared bank for QK/V/proj chains and transpose evacuation.
        tc.tile_pool(name="ps_e", bufs=3, space="PSUM") as ps_e,
        tc.tile_pool(name="ps_av", bufs=1, space="PSUM") as ps_av,
        tc.tile_pool(name="ps_m", bufs=1, space="PSUM") as ps_m,
    ):
        # ---- input DMAs. The DMA engines share one per-core bandwidth
        # pool (~350 GB/s), so what matters is BYTE order, not queue count:
        # the K^T(0)/Q^T(0) n=0 tiles only touch tokens 0..511, so x is
        # loaded in token-quarters and the first exp can start after just
        # wk + x-quarter0 + wq (~3 MB) instead of all 8 MB of inputs.
        x_s = sbp.tile([128, 8, T], bf16)
        xr = xT.rearrange("(m p) t -> p m t", p=128)
        wk_s = sbp.tile([128, 8, FH], bf16, tag="wk")
        wkr = wkT.rearrange("(m p) d -> p m d", p=128)
        wq_s = sbp.tile([128, 8, FH], bf16, tag="wq")
        wqr = wqT.rearrange("(m p) d -> p m d", p=128)
        # Only head-pair 0's slice of wk/wq (0.25 MB each) gates the first
        # exp; the other pairs' columns aren't read until the QK fillers
        # around unit 4, so they load after wv / the x quarters. This cuts
        # the DMA-bandwidth critical path to the first exp from ~3 MB to
        # ~1.5 MB and pulls wv (the V-projection gate) ~10 us earlier.
        nc.sync.dma_start(out=wk_s[:, :, 0:128], in_=wkr[:, :, 0:128])
        nc.scalar.dma_start(out=x_s[:, :, 0:512], in_=xr[:, :, 0:512])
        nc.sync.dma_start(out=wq_s[:, :, 0:128], in_=wqr[:, :, 0:128])
        bqkv_s = sbp.tile([128, 12], f32)
        nc.scalar.dma_start(out=bqkv_s[:], in_=bqkv)
        bqT_s = bqkv_s[:, 0:4]
        bkT_s = bqkv_s[:, 4:8]
        bvT_s = bqkv_s[:, 8:12]
        nc.scalar.dma_start(out=x_s[:, :, 512:1024], in_=xr[:, :, 512:1024])
        wv_s = sbp.tile([128, 8, FH], bf16, tag="wv")
        nc.sync.dma_start(out=wv_s[:], in_=wvT.rearrange("(m p) d -> p m d", p=128))
        nc.sync.dma_start(out=x_s[:, :, 1024:1536], in_=xr[:, :, 1024:1536])
        nc.scalar.dma_start(out=x_s[:, :, 1536:2048], in_=xr[:, :, 1536:2048])
        nc.sync.dma_start(out=wk_s[:, :, 128:512], in_=wkr[:, :, 128:512])
        nc.scalar.dma_start(out=wq_s[:, :, 128:512], in_=wqr[:, :, 128:512])
        id_s = sbp.tile([128, 128], bf16)
        nc.scalar.dma_start(out=id_s[:], in_=iden)
        wp_s = sbp.tile([128, 4, D], bf16)
        nc.scalar.dma_start(out=wp_s[:], in_=wpT.rearrange("(c p) o -> p c o", p=128))

        # QT/KT: [d-in-pair(128), head-pair(4), t]; V: [t-in-chunk(128),
        # t-chunk(16), head(8), 66] with col 64 = 1.0 (row-sum trick).
        QT_sb = qkv_sb.tile([128, 4, T], bf16)
        KT_sb = qkv_sb.tile([128, 4, T], bf16)
        V_sb = qkv_sb.tile([128, 16, NH_LOC, 65], bf16)
        nc.vector.memset(V_sb[:, :, :, 64:65], 1.0)
        # AttnOut^T (post-transpose): [f-in-chunk(128), f-chunk(4)=hp, t]
        AOT_sb = qkv_sb.tile([128, 4, T], bf16)

        def emit_qk_ntile(w_s, b_s, dst, hp, n, pool):
            # one n-tile of a Q^T/K^T projection: an 8-matmul chain + bias
            dsl = slice(hp * 128, (hp + 1) * 128)
            ps = pool.tile([128, 512], f32, tag="e" if pool is ps_e else "m",
                           name="qk_ps")
            for m in range(8):
                nc.tensor.matmul(ps[:], w_s[:, m, dsl],
                                 x_s[:, m, n * 512:(n + 1) * 512],
                                 start=(m == 0), stop=(m == 7))
            nc.vector.tensor_scalar_add(
                dst[:, hp, n * 512:(n + 1) * 512], ps[:], b_s[:, hp:hp + 1])

        def emit_v_tile(t, pool):
            # V (natural): out[t, f] = x[t, :].wvT[:, f]  (bias folded later)
            tag = {id(ps_e): "e", id(ps_av): "av", id(ps_m): "m"}[id(pool)]
            ps = pool.tile([128, 512], f32, tag=tag, name="v_ps")
            for m in range(8):
                nc.tensor.matmul(ps[:], x_s[:, m, t * 128:(t + 1) * 128],
                                 wv_s[:, m, :], start=(m == 0), stop=(m == 7))
            nc.vector.tensor_copy(
                V_sb[:, t, :, 0:64],
                ps[:].rearrange("p (h d) -> p h d", h=NH_LOC))

        def emit_proj(t, pool=None):
            # partial output projection (pre-bias) for token tile t
            pool = pool or ps_m
            tag = {id(ps_e): "e", id(ps_av): "av", id(ps_m): "m"}[id(pool)]
            tsl = slice(t * 128, (t + 1) * 128)
            st = ostage.tile([128, D], bf16, tag="st")
            for half in range(2):
                ps = pool.tile([128, 512], f32, tag=tag, name="pj")
                for fc in range(4):
                    nc.tensor.matmul(ps[:], AOT_sb[:, fc, tsl],
                                     wp_s[:, fc, half * 512:(half + 1) * 512],
                                     start=(fc == 0), stop=(fc == 3))
                nc.vector.tensor_copy(st[:, half * 512:(half + 1) * 512], ps[:])
            nc.sync.dma_start(out=out[tsl, :], in_=st[:])

        # ---- software-pipelined attention ----
        units = [(hp, j, s) for hp in range(4) for j in range(4)
                 for s in range(2)]
        state = {}      # u -> (pb, av) live tiles
        fillers = []    # queue of zero-arg emitters

        GROUPS = [(2 * g, 2 * g + 2) for g in range(8)]

        def emit_e_group(u, g):
            # fill one energies PSUM tile (3 key-chunks, 1 for the last
            # group) and exp it into pb
            hp, j, s = u
            psl = slice(64 * s, 64 * s + 64)
            qsl = slice(j * 512, (j + 1) * 512)
            pb = state[u][0][0] if g < 4 else state[u][0][1]
            lo, hi = GROUPS[g]
            off = 0 if g < 4 else 8
            w = hi - lo
            et = ps_e.tile([128, 2, 512], f32, tag="e", name="et")
            for i in range(w):
                kc = lo + i
                nc.tensor.matmul(et[:, i, :], KT_sb[psl, hp, kc * 128:(kc + 1) * 128],
                                 QT_sb[psl, hp, qsl], start=True, stop=True)
            nc.scalar.activation(pb[:, lo - off:hi - off, :], et[:, 0:w, :], AF.Exp)

        def emit_av_block(u, kcs, pool=None):
            # attn@V with pb as stationary: out[q, d] accumulates per
            # q-chunk; V col 64 (ones) accumulates the softmax row sums.
            hp, j, s = u
            h = 2 * hp + s
            (pba, pbb), av = state[u][0], state[u][1]
            if av is None:
                pool = pool or ps_av
                tag = {id(ps_e): "e", id(ps_av): "av", id(ps_m): "m"}[id(pool)]
                avt = pool.tile([128, 512], f32, tag=tag, name="avt")
                av = avt[:, 0:260].rearrange("p (q c) -> p q c", c=65)
                state[u] = ((pba, pbb), av, state[u][2])
            # One PSUM accumulation group for the WHOLE tile: start=True
            # zeroes the entire 2KB zero-region, so only the first matmul
            # emitted may carry it, and only the last carries stop (the
            # framework serializes accumulates into the same tile, so
            # emission order is execution order here).
            for qc in range(4):
                for kc in kcs:
                    pb = pba if kc < 8 else pbb
                    nc.tensor.matmul(
                        av[:, qc, :],
                        pb[:, kc % 8, qc * 128:(qc + 1) * 128],
                        V_sb[:, kc, h, 0:65],
                        start=(qc == 0 and kc == 0),
                        stop=(qc == 3 and kc == 15))

        def emit_norm(u, with_tr=False, with_proj=False):
            # softmax normalization: per-partition (per-query) reciprocal
            # of the row sums, then scale the 64 head dims into AO staging
            # [q, qsub, pair-features]. For s==1 units the per-q-chunk
            # transpose back to feature-major (+ V-bias add) is fused in so
            # the chain pipelines per chunk; for the final block each
            # chunk's projection tile follows immediately.
            hp, j, s = u
            av = state[u][1]
            ao = state[u][2]
            rr = rr_pool.tile([128, 4], f32, tag="rr", bufs=4)
            nc.vector.reciprocal(rr[:], av[:, :, 64])
            for qc in range(4):
                nc.vector.tensor_scalar_mul(
                    ao[:, qc, s * 64:(s + 1) * 64], av[:, qc, 0:64],
                    rr[:, qc:qc + 1])
                if with_tr:
                    tp = ps_m.tile([128, 128], bf16, tag="m", name="tp")
                    nc.tensor.transpose(tp[:], ao[:, qc, :], id_s[:])
                    nc.vector.tensor_scalar_add(
                        AOT_sb[:, hp,
                               j * 512 + qc * 128:j * 512 + (qc + 1) * 128],
                        tp[:], bvT_s[:, hp:hp + 1])
                    if with_proj:
                        emit_proj(4 * j + qc, ps_e)
            del state[u]

        def pop_filler():
            if fillers:
                fillers.pop(0)()

        # ---- prologue: K^T(0) fully and Q^T(0) n=0, pipelined through the
        # (still idle) energies PSUM rotation. The scheduler is readiness-
        # driven with emission order as priority, so unit 0's energy fills
        # (emitted next) start the exp stream as soon as these finish; the
        # V tiles are emitted just below unit 0 and flow through the misc
        # bank and the (pre-attn@V) accumulator bank in parallel.
        emit_qk_ntile(wk_s, bkT_s, KT_sb, 0, 0, ps_e)
        emit_qk_ntile(wq_s, bqT_s, QT_sb, 0, 0, ps_e)

        prev = None
        for ui, u in enumerate(units):
            hp, j, s = u
            if j == 2 and s == 0 and hp < 3:
                # queue the next pair's Q/K tiles late enough that they
                # don't compete with the V burst at the head
                for w_s, b_s, dst in ((wq_s, bqT_s, QT_sb),
                                      (wk_s, bkT_s, KT_sb)):
                    for n in range(4):
                        fillers.append(
                            lambda w=w_s, b=b_s, d=dst, p=hp + 1, nn=n:
                            emit_qk_ntile(w, b, d, p, nn, ps_m))
            if s == 0:
                pair_ao = ao_pool.tile([128, 4, 128], bf16, tag="ao",
                                       name="pair_ao")
            else:
                pair_ao = state[prev][2]
            pba = pb_pool.tile([128, 8, 512], bf16, tag="pbA", name="pba",
                               bufs=4)
            pbb = pb_pool.tile([128, 8, 512], bf16, tag="pbB", name="pbb",
                               bufs=5)
            state[u] = ((pba, pbb), None, pair_ao)
            # energy fills + exp first (highest priority: they feed the
            # Activation stream); attn@V / normalization / fillers behind -
            # the readiness-driven scheduler slots them into PSUM-wait gaps.
            for g in range(8):
                emit_e_group(u, g)
                if ui == 0 and g in (1, 3, 5):
                    # K^T(0) tile n = g//2 + 1 just ahead of the energy
                    # groups that need it (group g reads K^T n-tile g//2)
                    emit_qk_ntile(wk_s, bkT_s, KT_sb, 0, g // 2 + 1, ps_e)
                if ui == 0 and g in (2, 4, 6):
                    # Q^T(0) n=1..3 likewise through the energies rotation
                    emit_qk_ntile(wq_s, bqT_s, QT_sb, 0, g // 2, ps_e)
            if ui == 0:
                # All V tiles must be EMITTED before attn@V(u0) (dependency
                # edges only link to previously-emitted writers), but their
                # scheduling priority is demoted so they fill the PE's
                # exp-wait bubbles instead of starving the energy fills.
                with tc.high_priority(offset=-150):
                    for t in range(16):
                        emit_v_tile(t, ps_m if t % 2 else ps_av)
            if prev is not None:
                # previous unit's attn@V / normalization: lower priority
                # than this unit's fills; the scheduler slots them into
                # exp-wait gaps. Alternating the accumulator between the
                # attn@V bank and the misc bank decouples consecutive
                # units' av chains from each other's normalization reads.
                emit_av_block(prev, range(0, 8))
                emit_av_block(prev, range(8, 16))
                php, pj, ps_prev = prev
                emit_norm(prev, with_tr=(ps_prev == 1))
                if ps_prev == 1 and php == 3 and pj < 3:
                    for tt in range(4):
                        fillers.append(
                            lambda t=4 * pj + tt: emit_proj(t))
            pop_filler()
            pop_filler()
            prev = u
        # pipeline tail: last unit's attn@V through the (now idle)
        # energies rotation so it overlaps the final exps, then per-chunk
        # norm -> transpose -> projection
        emit_av_block(prev, range(0, 8), pool=ps_e)
        emit_av_block(prev, range(8, 16))
        emit_norm(prev, with_tr=True, with_proj=True)
        while fillers:
            pop_filler()


def get_program():
    if "nc" not in _prog_cache:
        _prog_cache["nc"] = _build_program()
    return _prog_cache["nc"]


def make_in_maps(inputs):
    x = np.asarray(inputs["x"], dtype=np.float32)
    Wq = np.asarray(inputs["Wq"], dtype=np.float32)
    bq = np.asarray(inputs["bq"], dtype=np.float32)
    Wk = np.asarray(inputs["Wk"], dtype=np.float32)
    bk = np.asarray(inputs["bk"], dtype=np.float32)
    Wv = np.asarray(inputs["Wv"], dtype=np.float32)
    bv = np.asarray(inputs["bv"], dtype=np.float32)
    Wp = np.asarray(inputs["Wp"], dtype=np.float32)

    iden = np.eye(128, dtype=np.float32).astype(BF16)
    in_maps = []
    for c in range(N_CORES):
        b, half = divmod(c, 2)
        fs = slice(half * FH, half * FH + FH)
        in_maps.append({
            "xT": np.ascontiguousarray(x[b].T).astype(BF16),
            "wqT": np.ascontiguousarray(Wq[fs].T).astype(BF16),
            "wkT": np.ascontiguousarray(Wk[fs].T).astype(BF16),
            "wvT": np.ascontiguousarray(Wv[fs].T).astype(BF16),
            "bqkv": np.ascontiguousarray(np.concatenate(
                [bq[fs].reshape(4, 128).T, bk[fs].reshape(4, 128).T,
                 bv[fs].reshape(4, 128).T], axis=1)),
            "wpT": np.ascontiguousarray(Wp[:, fs].T).astype(BF16),
            "iden": iden,
        })
    return in_maps


def gather_output(results, bp):
    bp = np.asarray(bp, dtype=np.float32)
    return np.stack([
        results[2 * b]["out"].astype(np.float32)
        + results[2 * b + 1]["out"].astype(np.float32) + bp[None, :]
        for b in range(4)
    ]).astype(np.float32)


def kernel(**inputs):
    nc = get_program()
    in_maps = make_in_maps(inputs)
    res = run_bass_kernel_spmd(nc, in_maps, list(range(N_CORES))).results
    return gather_output(res, inputs["bp"])


# revision 6
# speedup vs baseline: 1.0173x; 1.0023x over previous
"""Multi-head attention (16 heads, d_model=1024, head_dim=64) on 8 trn2 cores.

Sharding: core c handles batch b = c//2 and heads [8*(c%2), 8*(c%2)+8)
(data parallel over batch x tensor parallel over heads). Each core
computes its 8 heads' Q/K/V projections, attention, and a partial output
projection; the host sums the two partial projections per batch element
(the "all-reduce") and adds the output bias bp.

Key layout choices (all driven by PE cost ~ moving-operand columns):
- Q^T/K^T are produced feature-major [d, t] so energies contract along
  partitions (stationary K^T chunk [64, 128], moving Q^T [64, 512]).
- attn@V is computed with the EXP WEIGHTS AS STATIONARY ([128 keys, 128
  queries]) and V as the moving operand ([128 keys, 65]): each matmul
  streams only 65 columns (64 head dims + a ones-column that yields the
  softmax row-sums), 8x fewer moving columns than the [d, t]-stationary
  orientation. The attention output lands QUERY-major [q, d], where the
  softmax normalization is a cheap per-partition scalar multiply.
- Normalized [q, f] tiles are transposed back to feature-major via PE
  transpose (128 cycles per 128x128 tile) to feed the output projection;
  the V bias is folded into the transpose-evacuation copy (rows of the
  normalized attention matrix sum to 1, so A@(V + bv) = A@V + bv).

Softmax is unnormalized exp (no max subtraction; energies bounded ~|15|)
with row sums taken by the ones-column of V. exp runs on the Activation
engine in 2-key-chunk calls over a 3-deep rotation of [128,2,512] PSUM
tiles - the deepest rotation that fits PSUM (6 banks + 1 attn@V
accumulator bank + 1 projection/transpose scratch bank) so the exp
stream decouples from the PE's fill cadence. The tile scheduler is
readiness-driven with emission order as priority: energy fills/exps are
emitted first each unit, attn@V / normalization / filler projections
behind them, and the 16 V-projection tiles are emitted inside unit 0
(dependency edges only link to previously-emitted writers) at demoted
priority so they soak up PE slack across units 1-4.
"""

import numpy as np
import ml_dtypes

from concourse import bass, bacc, tile, mybir
from concourse.bass_utils import run_bass_kernel_spmd

BF16 = ml_dtypes.bfloat16
dt = mybir.dt
AF = mybir.ActivationFunctionType

N_CORES = 8
T = 2048          # tokens per batch element
D = 1024          # model dim
FH = 512          # features (head dims) per core: 8 heads x 64
NH_LOC = 8        # heads per core
HD = 64           # head dim

_prog_cache = {}


def _build_program():
    nc = bacc.Bacc("TRN2", target_bir_lowering=False, debug=False,
                   num_devices=N_CORES)

    xT = nc.dram_tensor("xT", [D, T], dt.bfloat16, kind="ExternalInput").ap()
    wqT = nc.dram_tensor("wqT", [D, FH], dt.bfloat16, kind="ExternalInput").ap()
    wkT = nc.dram_tensor("wkT", [D, FH], dt.bfloat16, kind="ExternalInput").ap()
    wvT = nc.dram_tensor("wvT", [D, FH], dt.bfloat16, kind="ExternalInput").ap()
    bqkv = nc.dram_tensor("bqkv", [128, 12], dt.float32, kind="ExternalInput").ap()
    wpT = nc.dram_tensor("wpT", [FH, D], dt.bfloat16, kind="ExternalInput").ap()
    iden = nc.dram_tensor("iden", [128, 128], dt.bfloat16, kind="ExternalInput").ap()
    out = nc.dram_tensor("out", [T, D], dt.bfloat16, kind="ExternalOutput").ap()

    with tile.TileContext(nc) as tc:
        _emit(tc, out, xT, wqT, wkT, wvT, bqkv, wpT, iden)
    nc.compile()
    return nc


def _emit(tc, out, xT, wqT, wkT, wvT, bqkv, wpT, iden):
    nc = tc.nc
    f32 = dt.float32
    bf16 = dt.bfloat16

    with (
        tc.tile_pool(name="sbp", bufs=1) as sbp,
        tc.tile_pool(name="qkv_sb", bufs=1) as qkv_sb,
        tc.tile_pool(name="pb_pool", bufs=4) as pb_pool,
        tc.tile_pool(name="ao_pool", bufs=1) as ao_pool,
        tc.tile_pool(name="rr_pool", bufs=2) as rr_pool,
        tc.tile_pool(name="ostage", bufs=3) as ostage,
        # PSUM budget (8 banks): energies 3x[128,2,512] (6 banks) rotating
        # under the exp stream; attn@V accumulator bank (also used by half
        # the V-projection tiles at the head, before attn@V starts); one
        # shared bank for QK/V/proj chains and transpose evacuation.
        tc.tile_pool(name="ps_e", bufs=3, space="PSUM") as ps_e,
        tc.tile_pool(name="ps_av", bufs=1, space="PSUM") as ps_av,
        tc.tile_pool(name="ps_m", bufs=1, space="PSUM") as ps_m,
    ):
        # ---- input DMAs. The DMA engines share one per-core bandwidth
        # pool (~350 GB/s), so what matters is BYTE order, not queue count:
        # the K^T(0)/Q^T(0) n=0 tiles only touch tokens 0..511, so x is
        # loaded in token-quarters and the first exp can start after just
        # wk + x-quarter0 + wq (~3 MB) instead of all 8 MB of inputs.
        x_s = sbp.tile([128, 8, T], bf16)
        xr = xT.rearrange("(m p) t -> p m t", p=128)
        wk_s = sbp.tile([128, 8, FH], bf16, tag="wk")
        wkr = wkT.rearrange("(m p) d -> p m d", p=128)
        wq_s = sbp.tile([128, 8, FH], bf16, tag="wq")
        wqr = wqT.rearrange("(m p) d -> p m d", p=128)
        # Only head-pair 0's slice of wk/wq (0.25 MB each) gates the first
        # exp; the other pairs' columns aren't read until the QK fillers
        # around unit 4, so they load after wv / the x quarters. This cuts
        # the DMA-bandwidth critical path to the first exp from ~3 MB to
        # ~1.5 MB and pulls wv (the V-projection gate) ~10 us earlier.
        nc.sync.dma_start(out=wk_s[:, :, 0:128], in_=wkr[:, :, 0:128])
        nc.scalar.dma_start(out=x_s[:, :, 0:512], in_=xr[:, :, 0:512])
        nc.sync.dma_start(out=wq_s[:, :, 0:128], in_=wqr[:, :, 0:128])
        bqkv_s = sbp.tile([128, 12], f32)
        nc.scalar.dma_start(out=bqkv_s[:], in_=bqkv)
        bqT_s = bqkv_s[:, 0:4]
        bkT_s = bqkv_s[:, 4:8]
        bvT_s = bqkv_s[:, 8:12]
        nc.scalar.dma_start(out=x_s[:, :, 512:1024], in_=xr[:, :, 512:1024])
        wv_s = sbp.tile([128, 8, FH], bf16, tag="wv")
        nc.sync.dma_start(out=wv_s[:], in_=wvT.rearrange("(m p) d -> p m d", p=128))
        nc.sync.dma_start(out=x_s[:, :, 1024:1536], in_=xr[:, :, 1024:1536])
        nc.scalar.dma_start(out=x_s[:, :, 1536:2048], in_=xr[:, :, 1536:2048])
        nc.sync.dma_start(out=wk_s[:, :, 128:512], in_=wkr[:, :, 128:512])
        nc.scalar.dma_start(out=wq_s[:, :, 128:512], in_=wqr[:, :, 128:512])
        id_s = sbp.tile([128, 128], bf16)
        nc.scalar.dma_start(out=id_s[:], in_=iden)
        wp_s = sbp.tile([128, 4, D], bf16)
        nc.scalar.dma_start(out=wp_s[:], in_=wpT.rearrange("(c p) o -> p c o", p=128))

        # QT/KT: [d-in-pair(128), head-pair(4), t]; V: [t-in-chunk(128),
        # t-chunk(16), head(8), 66] with col 64 = 1.0 (row-sum trick).
        QT_sb = qkv_sb.tile([128, 4, T], bf16)
        KT_sb = qkv_sb.tile([128, 4, T], bf16)
        V_sb = qkv_sb.tile([128, 16, NH_LOC, 65], bf16)
        nc.vector.memset(V_sb[:, :, :, 64:65], 1.0)
        # AttnOut^T (post-transpose): [f-in-chunk(128), f-chunk(4)=hp, t]
        AOT_sb = qkv_sb.tile([128, 4, T], bf16)

        def emit_qk_ntile(w_s, b_s, dst, hp, n, pool):
            # one n-tile of a Q^T/K^T projection: an 8-matmul chain + bias
            dsl = slice(hp * 128, (hp + 1) * 128)
            ps = pool.tile([128, 512], f32, tag="e" if pool is ps_e else "m",
                           name="qk_ps")
            for m in range(8):
                nc.tensor.matmul(ps[:], w_s[:, m, dsl],
                                 x_s[:, m, n * 512:(n + 1) * 512],
                                 start=(m == 0), stop=(m == 7))
            nc.vector.tensor_scalar_add(
                dst[:, hp, n * 512:(n + 1) * 512], ps[:], b_s[:, hp:hp + 1])

        def emit_v_tile(t, pool):
            # V (natural): out[t, f] = x[t, :].wvT[:, f]  (bias folded later)
            tag = {id(ps_e): "e", id(ps_av): "av", id(ps_m): "m"}[id(pool)]
            ps = pool.tile([128, 512], f32, tag=tag, name="v_ps")
            for m in range(8):
                nc.tensor.matmul(ps[:], x_s[:, m, t * 128:(t + 1) * 128],
                                 wv_s[:, m, :], start=(m == 0), stop=(m == 7))
            nc.vector.tensor_copy(
                V_sb[:, t, :, 0:64],
                ps[:].rearrange("p (h d) -> p h d", h=NH_LOC))

        def emit_proj(t, pool=None):
            # partial output projection (pre-bias) for token tile t
            pool = pool or ps_m
            tag = {id(ps_e): "e", id(ps_av): "av", id(ps_m): "m"}[id(pool)]
            tsl = slice(t * 128, (t + 1) * 128)
            st = ostage.tile([128, D], bf16, tag="st")
            for half in range(2):
                ps = pool.tile([128, 512], f32, tag=tag, name="pj")
                for fc in range(4):
                    nc.tensor.matmul(ps[:], AOT_sb[:, fc, tsl],
                                     wp_s[:, fc, half * 512:(half + 1) * 512],
                                     start=(fc == 0), stop=(fc == 3))
                nc.vector.tensor_copy(st[:, half * 512:(half + 1) * 512], ps[:])
            nc.sync.dma_start(out=out[tsl, :], in_=st[:])

        # ---- software-pipelined attention ----
        units = [(hp, j, s) for hp in range(4) for j in range(4)
                 for s in range(2)]
        state = {}      # u -> (pb, av) live tiles
        fillers = []    # queue of zero-arg emitters

        GROUPS = [(2 * g, 2 * g + 2) for g in range(8)]

        def emit_e_group(u, g):
            # fill one energies PSUM tile (3 key-chunks, 1 for the last
            # group) and exp it into pb
            hp, j, s = u
            psl = slice(64 * s, 64 * s + 64)
            qsl = slice(j * 512, (j + 1) * 512)
            pb = state[u][0][0] if g < 4 else state[u][0][1]
            lo, hi = GROUPS[g]
            off = 0 if g < 4 else 8
            w = hi - lo
            et = ps_e.tile([128, 2, 512], f32, tag="e", name="et")
            for i in range(w):
                kc = lo + i
                nc.tensor.matmul(et[:, i, :], KT_sb[psl, hp, kc * 128:(kc + 1) * 128],
                                 QT_sb[psl, hp, qsl], start=True, stop=True)
            nc.scalar.activation(pb[:, lo - off:hi - off, :], et[:, 0:w, :], AF.Exp)

        def emit_av_block(u, kcs, pool=None):
            # attn@V with pb as stationary: out[q, d] accumulates per
            # q-chunk; V col 64 (ones) accumulates the softmax row sums.
            hp, j, s = u
            h = 2 * hp + s
            (pba, pbb), av = state[u][0], state[u][1]
            if av is None:
                pool = pool or ps_av
                tag = {id(ps_e): "e", id(ps_av): "av", id(ps_m): "m"}[id(pool)]
                avt = pool.tile([128, 512], f32, tag=tag, name="avt")
                av = avt[:, 0:260].rearrange("p (q c) -> p q c", c=65)
                state[u] = ((pba, pbb), av, state[u][2])
            # One PSUM accumulation group for the WHOLE tile: start=True
            # zeroes the entire 2KB zero-region, so only the first matmul
            # emitted may carry it, and only the last carries stop (the
            # framework serializes accumulates into the same tile, so
            # emission order is execution order here).
            for qc in range(4):
                for kc in kcs:
                    pb = pba if kc < 8 else pbb
                    nc.tensor.matmul(
                        av[:, qc, :],
                        pb[:, kc % 8, qc * 128:(qc + 1) * 128],
                        V_sb[:, kc, h, 0:65],
                        start=(qc == 0 and kc == 0),
                        stop=(qc == 3 and kc == 15))

        def emit_norm(u, with_tr=False, with_proj=False):
            # softmax normalization: per-partition (per-query) reciprocal
            # of the row sums, then scale the 64 head dims into AO staging
            # [q, qsub, pair-features]. For s==1 units the per-q-chunk
            # transpose back to feature-major (+ V-bias add) is fused in so
            # the chain pipelines per chunk; for the final block each
            # chunk's projection tile follows immediately.
            hp, j, s = u
            av = state[u][1]
            ao = state[u][2]
            rr = rr_pool.tile([128, 4], f32, tag="rr", bufs=4)
            nc.vector.reciprocal(rr[:], av[:, :, 64])
            for qc in range(4):
                nc.vector.tensor_scalar_mul(
                    ao[:, qc, s * 64:(s + 1) * 64], av[:, qc, 0:64],
                    rr[:, qc:qc + 1])
                if with_tr:
                    tp = ps_m.tile([128, 128], bf16, tag="m", name="tp")
                    nc.tensor.transpose(tp[:], ao[:, qc, :], id_s[:])
                    nc.vector.tensor_scalar_add(
                        AOT_sb[:, hp,
                               j * 512 + qc * 128:j * 512 + (qc + 1) * 128],
                        tp[:], bvT_s[:, hp:hp + 1])
                    if with_proj:
                        emit_proj(4 * j + qc, ps_e)
            del state[u]

        def pop_filler():
            if fillers:
                fillers.pop(0)()

        # ---- prologue: K^T(0) fully and Q^T(0) n=0, pipelined through the
        # (still idle) energies PSUM rotation. The scheduler is readiness-
        # driven with emission order as priority, so unit 0's energy fills
        # (emitted next) start the exp stream as soon as these finish; the
        # V tiles are emitted just below unit 0 and flow through the misc
        # bank and the (pre-attn@V) accumulator bank in parallel.
        emit_qk_ntile(wk_s, bkT_s, KT_sb, 0, 0, ps_e)
        emit_qk_ntile(wq_s, bqT_s, QT_sb, 0, 0, ps_e)

        prev = None
        for ui, u in enumerate(units):
            hp, j, s = u
            if j == 2 and s == 0 and hp < 3:
                # queue the next pair's Q/K tiles late enough that they
                # don't compete with the V burst at the head
                for w_s, b_s, dst in ((wq_s, bqT_s, QT_sb),
                                      (wk_s, bkT_s, KT_sb)):
                    for n in range(4):
                        fillers.append(
                            lambda w=w_s, b=b_s, d=dst, p=hp + 1, nn=n:
                            emit_qk_ntile(w, b, d, p, nn, ps_m))
            if s == 0:
                pair_ao = ao_pool.tile([128, 4, 128], bf16, tag="ao",
                                       name="pair_ao")
            else:
                pair_ao = state[prev][2]
            pba = pb_pool.tile([128, 8, 512], bf16, tag="pbA", name="pba",
                               bufs=4)
            pbb = pb_pool.tile([128, 8, 512], bf16, tag="pbB", name="pbb",
                               bufs=5)
            state[u] = ((pba, pbb), None, pair_ao)
            # energy fills + exp first (highest priority: they feed the
            # Activation stream); attn@V / normalization / fillers behind -
            # the readiness-driven scheduler slots them into PSUM-wait gaps.
            for g in range(8):
                emit_e_group(u, g)
                if ui == 0 and g in (1, 3, 5):
                    # K^T(0) tile n = g//2 + 1 just ahead of the energy
                    # groups that need it (group g reads K^T n-tile g//2)
                    emit_qk_ntile(wk_s, bkT_s, KT_sb, 0, g // 2 + 1, ps_e)
                if ui == 0 and g in (2, 4, 6):
                    # Q^T(0) n=1..3 likewise through the energies rotation
                    emit_qk_ntile(wq_s, bqT_s, QT_sb, 0, g // 2, ps_e)
            if ui == 0:
                # All V tiles must be EMITTED before attn@V(u0) (dependency
                # edges only link to previously-emitted writers), but their
                # scheduling priority is demoted so they fill the PE's
                # exp-wait bubbles instead of starving the energy fills.
                with tc.high_priority(offset=-150):
                    for t in range(16):
                        emit_v_tile(t, ps_m if t % 2 else ps_av)
            if prev is not None:
                # previous unit's attn@V / normalization: lower priority
                # than this unit's fills; the scheduler slots them into
                # exp-wait gaps. Alternating the accumulator between the
                # attn@V bank and the misc bank decouples consecutive
                # units' av chains from each other's normalization reads.
                emit_av_block(prev, range(0, 8))
                emit_av_block(prev, range(8, 16))
                php, pj, ps_prev = prev
                emit_norm(prev, with_tr=(ps_prev == 1))
                if ps_prev == 1 and php == 3 and pj < 3:
                    for tt in range(4):
                        fillers.append(
                            lambda t=4 * pj + tt: emit_proj(t))
            pop_filler()
            pop_filler()
            prev = u
        # pipeline tail: last unit's attn@V through the (now idle)
        # energies rotation so it overlaps the final exps, then per-chunk
        # norm -> transpose -> projection
        emit_av_block(prev, range(0, 8), pool=ps_e)
        emit_av_block(prev, range(8, 16))
        emit_norm(prev, with_tr=True, with_proj=True)
        while fillers:
            pop_filler()


def get_program():
    if "nc" not in _prog_cache:
        _prog_cache["nc"] = _build_program()
    return _prog_cache["nc"]


def make_in_maps(inputs):
    x = np.asarray(inputs["x"], dtype=np.float32)
    Wq = np.asarray(inputs["Wq"], dtype=np.float32)
    bq = np.asarray(inputs["bq"], dtype=np.float32)
    Wk = np.asarray(inputs["Wk"], dtype=np.float32)
    bk = np.asarray(inputs["bk"], dtype=np.float32)
    Wv = np.asarray(inputs["Wv"], dtype=np.float32)
    bv = np.asarray(inputs["bv"], dtype=np.float32)
    Wp = np.asarray(inputs["Wp"], dtype=np.float32)

    iden = np.eye(128, dtype=np.float32).astype(BF16)
    in_maps = []
    for c in range(N_CORES):
        b, half = divmod(c, 2)
        fs = slice(half * FH, half * FH + FH)
        in_maps.append({
            "xT": np.ascontiguousarray(x[b].T).astype(BF16),
            "wqT": np.ascontiguousarray(Wq[fs].T).astype(BF16),
            "wkT": np.ascontiguousarray(Wk[fs].T).astype(BF16),
            "wvT": np.ascontiguousarray(Wv[fs].T).astype(BF16),
            "bqkv": np.ascontiguousarray(np.concatenate(
                [bq[fs].reshape(4, 128).T, bk[fs].reshape(4, 128).T,
                 bv[fs].reshape(4, 128).T], axis=1)),
            "wpT": np.ascontiguousarray(Wp[:, fs].T).astype(BF16),
            "iden": iden,
        })
    return in_maps


def gather_output(results, bp):
    bp = np.asarray(bp, dtype=np.float32)
    return np.stack([
        results[2 * b]["out"].astype(np.float32)
        + results[2 * b + 1]["out"].astype(np.float32) + bp[None, :]
        for b in range(4)
    ]).astype(np.float32)


def kernel(**inputs):
    nc = get_program()
    in_maps = make_in_maps(inputs)
    res = run_bass_kernel_spmd(nc, in_maps, list(range(N_CORES))).results
    return gather_output(res, inputs["bp"])
